# revision 3
# baseline (speedup 1.0000x reference)
import numpy as np
import jax
import jax.numpy as jnp
from functools import partial

# Hardcoded problem config (nn_LNO2d): b=2, s=256
WIDTH = 64
RANK = 4
CLEVEL = 1
MLEVEL = 2
NB = 4
HID1, HID2 = 64, 128
B, S = 2, 256
NDEV = 8

PARAM_NAMES = [
    "pW", "pb", "q1W", "q1b", "q2W", "q2b",
    "phiW1", "phib1", "phiW2", "phib2", "phiW3", "phib3",
    "psiW1", "psib1", "psiW2", "psib2", "psiW3", "psib3",
    "lnG", "lnB", "convW", "convb",
]


def _mlp3(x, W1, b1, W2, b2, W3, b3):
    h = jax.nn.relu(x @ W1 + b1)
    h = jax.nn.relu(h @ W2 + b2)
    return h @ W3 + b3


def _lowrank(v, a, phi_p, psi_p):
    b, sx, sy, w = v.shape
    n = sx * sy
    af = a.reshape(b, n, 3)
    vf = v.reshape(b, n, w)
    phi = _mlp3(af, *phi_p).reshape(b, n, w, RANK)
    psi = _mlp3(af, *psi_p).reshape(b, n, w, RANK)
    y = jnp.einsum("bnwr,bnw->bwr", psi, vf) / n
    out = jnp.einsum("bnwr,bwr->bnw", phi, y)
    return out.reshape(b, sx, sy, w)


def _conv3x3(x, W, bias):
    y = jax.lax.conv_general_dilated(x, W, (1, 1), "SAME",
                                     dimension_numbers=("NCHW", "OIHW", "NCHW"))
    return y + bias[None, :, None, None]


def _up_axis(x, f, axis):
    # gather-free bilinear upsample by integer factor f along `axis`
    # (half-pixel centers, edge-clamped == jax.image.resize bilinear upscale)
    n = x.shape[axis]

    def shift(src, d):
        # src shifted by d with edge replication, along `axis`
        if d == -1:
            sl = [slice(None)] * x.ndim
            sl[axis] = slice(0, 1)
            first = src[tuple(sl)]
            sl[axis] = slice(0, n - 1)
            return jnp.concatenate([first, src[tuple(sl)]], axis=axis)
        else:  # d == +1
            sl = [slice(None)] * x.ndim
            sl[axis] = slice(1, n)
            rest = src[tuple(sl)]
            sl[axis] = slice(n - 1, n)
            return jnp.concatenate([rest, src[tuple(sl)]], axis=axis)

    prev, nxt = shift(x, -1), shift(x, 1)
    if f == 2:
        phases = [0.75 * x + 0.25 * prev, 0.75 * x + 0.25 * nxt]
    elif f == 4:
        phases = [0.625 * x + 0.375 * prev, 0.875 * x + 0.125 * prev,
                  0.875 * x + 0.125 * nxt, 0.625 * x + 0.375 * nxt]
    else:
        raise ValueError(f)
    stacked = jnp.stack(phases, axis=axis + 1)  # (..., n, f, ...)
    new_shape = list(x.shape)
    new_shape[axis] = n * f
    return stacked.reshape(new_shape)


def _upsample(x, f):
    # x: (b, c, h, w) -> (b, c, f*h, f*w)
    return _up_axis(_up_axis(x, f, 2), f, 3)


def _local_correction(xc, Ws, bs):
    out = _conv3x3(xc, Ws[0], bs[0])
    for l in range(1, Ws.shape[0]):
        xl = xc[:, :, :: 2 ** l, :: 2 ** l]
        yl = _conv3x3(xl, Ws[l], bs[l])
        out = out + _upsample(yl, 2 ** l)
    return out


def _layernorm(x, g, b, eps=1e-5):
    mu = jnp.mean(x, axis=-1, keepdims=True)
    var = jnp.mean(jnp.square(x - mu), axis=-1, keepdims=True)
    return (x - mu) * jax.lax.rsqrt(var + eps) * g + b


def _forward(x, a, p):
    # x: (1, s, s, 1), a: (1, s, s, 2) -- one image
    b, sx, sy, _ = x.shape
    xa = jnp.concatenate([a, x], axis=-1)
    acat = xa
    h = xa @ p["pW"] + p["pb"]
    c = 2 ** CLEVEL
    for i in range(NB):
        x1 = _local_correction(h.transpose(0, 3, 1, 2), p["convW"][i],
                               p["convb"][i]).transpose(0, 2, 3, 1)
        hd = h[:, ::c, ::c]
        ad = acat[:, ::c, ::c]
        x2 = _lowrank(hd, ad,
                      (p["phiW1"][i], p["phib1"][i], p["phiW2"][i],
                       p["phib2"][i], p["phiW3"][i], p["phib3"][i]),
                      (p["psiW1"][i], p["psib1"][i], p["psiW2"][i],
                       p["psib2"][i], p["psiW3"][i], p["psib3"][i]))
        x2 = _upsample(x2.transpose(0, 3, 1, 2), c).transpose(0, 2, 3, 1)
        h = _layernorm(x1 + x2, p["lnG"][i], p["lnB"][i])
        if i != NB - 1:
            h = jax.nn.relu(h)
    h1 = jax.nn.relu(h @ p["q1W"] + p["q1b"])
    return h1 @ p["q2W"] + p["q2b"]


@partial(jax.pmap, axis_name="d")
def _pmapped(x, a, params):
    # each device: full forward on its image, return its row-quarter
    out = _forward(x, a, params)[0]  # (s, s, 1)
    q = jax.lax.axis_index("d") % 4
    return jax.lax.dynamic_slice_in_dim(out, q * (S // 4), S // 4, axis=0)


_compiled = False


def kernel(**inputs):
    global _compiled
    x = np.asarray(inputs["x"])
    a = np.asarray(inputs["a"])
    # device i handles image i//4, outputs row-quarter i%4
    xs = np.stack([x[i // 4] for i in range(NDEV)])[:, None]  # (8,1,s,s,1)
    as_ = np.stack([a[i // 4] for i in range(NDEV)])[:, None]
    params = {k: np.broadcast_to(np.asarray(inputs[k]),
                                 (NDEV,) + np.asarray(inputs[k]).shape)
              for k in PARAM_NAMES}
    quarters = _pmapped(jnp.asarray(xs), jnp.asarray(as_), params)
    quarters = np.asarray(quarters)  # (8, s/4, s, 1)
    out = np.empty((B, S, S, 1), np.float32)
    for i in range(NDEV):
        b, q = i // 4, i % 4
        out[b, q * (S // 4):(q + 1) * (S // 4)] = quarters[i]
    _compiled = True
    return out
